# revision 100
# baseline (speedup 1.0000x reference)
"""AttentionMIL forward on 8 Trainium2 NeuronCores.

Data-parallel over the 16 bags (2 bags/core). Per bag:
  h1 = relu(LN(x @ W1 + b1))          x:[4096,1024] W1:[1024,512]
  h2 = relu(LN(h1 @ W2 + b2))
  s  = tanh(h2 @ Wa1 + ba1) @ wa2     (+ba2 dropped: softmax shift-invariant)
  attn = softmax(s); pooled = sum(attn * h2)
  logits = relu(pooled @ Wc1 + bc1) @ Wc2 + bc2

Matmuls run in float32r (reduced-precision fp32, ~4e-4 rel err, 4x faster
than fp32 on the PE). Tokens live on the partition axis so LN/softmax
reductions are free-axis ops; x is pre-transposed on the host so the only
on-device transposes are h1/h2 (PE transpose-mode).

Fast path (biases zero, gammas one — exactly what setup_inputs produces)
exploits LN scale-invariance: relu commutes with the positive rstd, LN2
cancels LN1's rstd entirely, and LN2's rstd is deferred into the tanh
scale and the attention weights. This keeps ACT pinned to one activation
table (Copy/Relu/Tanh/Exp) except one batched Sqrt per bag.
"""

import numpy as np

B, N, D, H, C = 16, 4096, 1024, 512, 2
NCORES = 8
NB = B // NCORES       # bags per core
P = 128
NT = N // P            # token tiles per bag
DK = D // P            # k-chunks for D
HK = H // P            # k-chunks for H

_BUILD_CACHE = {}


def _build(flags):
    import concourse.bass as bass
    import concourse.mybir as mybir
    import concourse.tile as tile
    import concourse.bass_isa as bass_isa
    from concourse import bacc
    from concourse.masks import make_identity
    import contextlib

    z_b1, aff1, z_b2, aff2, z_ba1, safe_exp = flags
    fast = z_b1 and z_b2 and z_ba1 and not aff1 and not aff2 and safe_exp
    f32 = mybir.dt.float32
    f32r = mybir.dt.float32r
    EPS = 1e-5

    nc = bacc.Bacc(None, target_bir_lowering=False, debug=False)

    # f32r DRAM declarations: DMA loads straight into f32r SBUF tiles
    # (hardware rounds on PE read; verified bit-compatible with np.float32).
    bf16 = mybir.dt.bfloat16
    f8 = mybir.dt.float8e4
    DRm = mybir.MatmulPerfMode.DoubleRow
    # fast path: the three big GEMMs run in fp8e4m3 with DoubleRow perf mode
    # (2 k-tiles per instruction, 0.5 cycles/row — 4x the f32r rate).
    # Host pre-scales: W1c*16, W2c*16, Wa1*16 (powers of 2 that the
    # scale-invariant LN chain and the deferred rstd2 absorb exactly).
    # h1/h2 activations are bf16; transposes run on bf16 data (1.0 cyc/row).
    # walrus forbids mixing 32-bit and non-32-bit matmul inputs, so each
    # matmul's operand pair switches dtype together.
    wdt = f8 if fast else f32r
    xt = nc.dram_tensor("xt", [NB, D, N], wdt, kind="ExternalInput")
    W1 = nc.dram_tensor("W1", [D, H], wdt, kind="ExternalInput")
    W2 = nc.dram_tensor("W2", [H, H], wdt, kind="ExternalInput")
    Wa1 = nc.dram_tensor("Wa1", [H, H], wdt, kind="ExternalInput")
    W1s = W2s = Wa1s = None
    if fast:
        W1s = nc.dram_tensor("W1s", [D, H], f8, kind="ExternalInput")
        W2s = nc.dram_tensor("W2s", [H, H], f8, kind="ExternalInput")
        Wa1s = nc.dram_tensor("Wa1s", [H, H], f8, kind="ExternalInput")
    wa2 = nc.dram_tensor("wa2", [H, 1], f32, kind="ExternalInput")
    Wc1 = nc.dram_tensor("Wc1", [H, H], f32, kind="ExternalInput")
    Wc2 = nc.dram_tensor("Wc2", [H, C], f32, kind="ExternalInput")
    bc1 = nc.dram_tensor("bc1", [H], f32, kind="ExternalInput")
    bc2 = nc.dram_tensor("bc2", [C], f32, kind="ExternalInput")
    b1 = g1 = be1 = b2 = g2 = be2 = ba1 = None
    if not z_b1:
        b1 = nc.dram_tensor("b1", [H], f32, kind="ExternalInput")
    if aff1:
        g1 = nc.dram_tensor("g1", [H], f32, kind="ExternalInput")
        be1 = nc.dram_tensor("beta1", [H], f32, kind="ExternalInput")
    if not z_b2:
        b2 = nc.dram_tensor("b2", [H], f32, kind="ExternalInput")
    if aff2:
        g2 = nc.dram_tensor("g2", [H], f32, kind="ExternalInput")
        be2 = nc.dram_tensor("beta2", [H], f32, kind="ExternalInput")
    if not z_ba1:
        ba1 = nc.dram_tensor("ba1", [H], f32, kind="ExternalInput")
    y = nc.dram_tensor("y", [NB, C], f32, kind="ExternalOutput")

    AX = mybir.AxisListType
    OP = mybir.AluOpType
    AF = mybir.ActivationFunctionType

    with tile.TileContext(nc) as tc:
        ctx = contextlib.ExitStack()
        with ctx:
            wpool = ctx.enter_context(tc.tile_pool(name="wpool", bufs=1))
            xtr = ctx.enter_context(tc.tile_pool(name="xtr", bufs=4))
            h1tp = ctx.enter_context(tc.tile_pool(name="h1tp", bufs=2))
            h1p = ctx.enter_context(tc.tile_pool(name="h1p", bufs=3))
            htp = ctx.enter_context(tc.tile_pool(name="htp", bufs=4))
            ap_ = ctx.enter_context(tc.tile_pool(name="ap_", bufs=3))
            # 2 bags of h2 tiles coexist: bag b's h2 is pooled only at the
            # end of the CD|AB brace, while bag b+1's AB phase is already
            # writing its own h2 tiles.
            h2p = ctx.enter_context(tc.tile_pool(name="h2p", bufs=2 * NT))
            stats = ctx.enter_context(tc.tile_pool(name="stats", bufs=8))
            smallp = ctx.enter_context(tc.tile_pool(name="smallp", bufs=2))
            # "mm" ring (ps2/psa) and the mm1 "z1" ring are separate tags:
            # sharing one 4-ring lets the 4-wide z1 bursts at supertile
            # boundaries cycle the ring past ps2/psa slots, coupling mm1's
            # start to unrelated consumers. 2+2 banks, same footprint.
            psmm = ctx.enter_context(tc.tile_pool(name="psmm", bufs=2, space="PSUM"))
            pstr = ctx.enter_context(tc.tile_pool(name="pstr", bufs=3, space="PSUM"))
            # pooled accumulator gets its own PSUM bank: its accumulation
            # group must not share a 2KB zero-region with transpose tiles
            # (their start=True marks would wipe in-flight pooled columns).
            pspool = ctx.enter_context(tc.tile_pool(name="pspool", bufs=1, space="PSUM"))

            # ---- one-time init: identities, eps, weights in f32r ----
            ident_f = wpool.tile([P, P], f32)
            make_identity(nc, ident_f)
            ident_r = wpool.tile([P, P], f32r)
            nc.vector.tensor_copy(ident_r[:], ident_f[:])
            # bf16 identity: transpose cost in the PE keys off the moving
            # (identity) operand's dtype — bf16 runs 1.0 cycles/row vs 1.5
            # for f32r, with bit-exact results (0/1 exact in bf16).
            bf16 = mybir.dt.bfloat16
            ident_h = wpool.tile([P, P], bf16)
            nc.vector.tensor_copy(ident_h[:], ident_f[:])
            eps_t = wpool.tile([P, 1], f32)
            nc.vector.memset(eps_t, EPS)
            # fast path: ps2 = 256*z2c (W1c,W2c both x16), so
            # sqrt(V/256 + 256*eps) = 16*sqrt(var+eps) and R = rstd2/16.
            eps256_t = wpool.tile([P, 1], f32)
            nc.vector.memset(eps256_t, 256.0 * EPS)

            def load_conv(dram_ap, shape, name, eng=None):
                cv = wpool.tile(list(shape), f32r, name=f"wr_{name}")
                (eng or nc.sync).dma_start(cv[:], dram_ap)
                return cv

            # weight tiles are allocated up front; their DMAs are emitted
            # lazily (fast path) so the xt tile stream isn't starved at start.
            w1r = wpool.tile([P, DK, H], wdt, name="wr_w1")
            w2r = wpool.tile([P, HK, H], wdt, name="wr_w2")
            war = wpool.tile([P, HK, H], wdt, name="wr_wa1")
            # fp8 residual planes: W_scaled - fp8(W_scaled), representable
            # directly in e4m3 (subnormals). A second DR pass per GEMM
            # accumulates them, cutting the systematic weight-quantization
            # error ~18x for only PE cycles (PE has headroom under ACT).
            if fast:
                w1s = wpool.tile([P, DK, H], f8, name="wr_w1s")
                w2s = wpool.tile([P, HK, H], f8, name="wr_w2s")
                was = wpool.tile([P, HK, H], f8, name="wr_wa1s")
            wc1r = wpool.tile([P, HK, HK, P], f32, name="wr_wc1")
            wc2r = wpool.tile([P, HK, C], f32, name="wr_wc2")
            wa2_rep = wpool.tile([P, H], f32)
            # bf16 copy of the broadcast wa2 row: the s-dot multiply runs on
            # DVE in all-16-bit mode (2x) instead of the slow gpsimd engine,
            # which paces the score phase otherwise.
            wa2_rep_h = wpool.tile([P, H], bf16)
            bc1t = wpool.tile([P, HK], f32)
            bc2t = wpool.tile([C, 1], f32)

            def emit_w1(lo, hi):
                _w1p = W1.rearrange("(k p) h -> p k h", p=P)
                for _k in range(lo, hi):
                    nc.sync.dma_start(
                        w1r[:, _k : _k + 1, :], _w1p[:, _k : _k + 1, :]
                    )

            def emit_w1s():
                nc.sync.dma_start(w1s[:], W1s.rearrange("(k p) h -> p k h", p=P))

            def emit_w2():
                nc.sync.dma_start(w2r[:], W2.rearrange("(k p) h -> p k h", p=P))
                if fast:
                    nc.sync.dma_start(
                        w2s[:], W2s.rearrange("(k p) h -> p k h", p=P)
                    )

            def emit_wa1():
                nc.sync.dma_start(war[:], Wa1.rearrange("(k p) h -> p k h", p=P))
                if fast:
                    nc.sync.dma_start(
                        was[:], Wa1s.rearrange("(k p) h -> p k h", p=P)
                    )

            def emit_wcls():
                nc.sync.dma_start(
                    wc1r[:], Wc1.rearrange("(k p) (m j) -> p k m j", p=P, j=P)
                )
                nc.sync.dma_start(wc2r[:], Wc2.rearrange("(k p) c -> p k c", p=P))
                nc.sync.dma_start(bc1t[:], bc1.rearrange("(m p) -> p m", p=P))
                nc.sync.dma_start(bc2t[:], bc2[:, None])
                nc.gpsimd.dma_start(
                    wa2_rep[:], wa2.rearrange("h 1 -> 1 h").to_broadcast((P, H))
                )
                nc.vector.tensor_copy(wa2_rep_h[:], wa2_rep[:])

            def rep(v, name):
                if v is None:
                    return None
                t = wpool.tile([P, H], f32, name=f"rep_{name}")
                nc.gpsimd.dma_start(t[:], v[None, :].to_broadcast((P, H)))
                return t

            b1_rep = rep(b1, "b1")
            g1_rep = rep(g1, "g1")
            be1_rep = rep(be1, "be1")
            b2_rep = rep(b2, "b2")
            g2_rep = rep(g2, "g2")
            be2_rep = rep(be2, "be2")
            ba1_rep = rep(ba1, "ba1")

            if not fast:
                emit_w1(0, DK)
                emit_w2()
                emit_wa1()
                emit_wcls()

            xt_part = xt.rearrange("b (k p) n -> b p k n", p=P)

            # ---- general-path layernorm: full stats, relu-apply into out --
            def ln_relu(src_ps, out_sb, b_rep, g_rep, be_rep, tag):
                if b_rep is not None:
                    t = ap_.tile([P, H], f32, tag=f"lnb_{tag}", name=f"lnb_{tag}")
                    nc.vector.tensor_add(t[:], src_ps[:], b_rep[:])
                    src = t
                else:
                    src = src_ps
                bn = stats.tile([P, 6], f32, tag="bn", name="bn")
                nc.vector.bn_stats(bn[:], src[:])
                mv = stats.tile([P, 2], f32, tag="mv", name="mv")
                nc.vector.bn_aggr(mv[:], bn[:])
                sd = stats.tile([P, 1], f32, tag="sd", name="sd")
                nc.scalar.activation(sd[:], mv[:, 1:2], AF.Sqrt, bias=eps_t[:])
                rstd = stats.tile([P, 1], f32, tag="rstd", name="rstd")
                nc.vector.reciprocal(rstd[:], sd[:])
                nmr = stats.tile([P, 1], f32, tag="nmr", name="nmr")
                nc.vector.tensor_scalar(
                    nmr[:], mv[:, 0:1], rstd[:], -1.0, op0=OP.mult, op1=OP.mult
                )
                if g_rep is None:
                    nc.scalar.activation(
                        out_sb, src[:], AF.Relu, bias=nmr[:], scale=rstd[:]
                    )
                else:
                    z = ap_.tile([P, H], f32, tag=f"lnz_{tag}", name=f"lnz_{tag}")
                    nc.vector.tensor_scalar(
                        z[:], src[:], mv[:, 0:1], rstd[:], op0=OP.subtract, op1=OP.mult
                    )
                    nc.vector.tensor_mul(z[:], z[:], g_rep[:])
                    nc.vector.tensor_add(z[:], z[:], be_rep[:])
                    nc.scalar.activation(out_sb, z[:], AF.Relu)

            poolT_sb = smallp.tile([P, HK, NB], f32, bufs=1)

            # per-bag state shared between emission stages
            bag_state = [dict() for _ in range(NB)]

            XS = 4  # tokens per mm1 block: 512 = max fp8 DR moving width / 2
            SUP = NT // XS

            def prefetch_super(b, s):
                """Issue the xt DMA for supertile s (XS*P tokens) of bag b."""
                st = bag_state[b]
                xt_r = xtr.tile([P, DK, XS * P], wdt, tag="xtr", name="xt_r")
                nc.sync.dma_start(
                    xt_r[:], xt_part[b, :, :, s * XS * P : (s + 1) * XS * P]
                )
                st.setdefault("xt_map", {})[s] = xt_r

            def stage_A(b, s):
                """feature-layout mm1 for one supertile: out z1T[h-chunk,
                tokens] via stationary W1c chunks, so relu writes h1
                TRANSPOSED directly — the h1 PE-transpose and PSUM->SBUF
                copy of the token-layout version disappear.

                W1 is column-centered on the host, so x @ W1c directly
                yields z1 - mean_h(z1): LN1's mean subtraction is free and
                LN2 cancels the (positive) rstd1, leaving just relu.
                """
                st = bag_state[b]
                xt_r = st["xt_map"].pop(s)
                span = slice(s * XS * P, (s + 1) * XS * P)
                for c in range(HK):
                    z1 = psmm.tile([P, XS * P], f32, tag="z1", name="z1T", bufs=2)
                    for w_, last in ((w1r, False), (w1s, True)):
                        for k in range(DK // 2):
                            nc.tensor.matmul(
                                z1[:],
                                w_[:, 2 * k : 2 * k + 2, c * P : (c + 1) * P],
                                xt_r[:, 2 * k : 2 * k + 2, :],
                                start=(w_ is w1r and k == 0),
                                stop=(last and k == DK // 2 - 1),
                                perf_mode=DRm,
                            )
                    dst = st["h1T"][:, c, span]
                    if c % 2 == 0:
                        nc.scalar.activation(dst, z1[:], AF.Relu)
                    else:
                        nc.vector.tensor_scalar(
                            dst, z1[:], 0.0, None, op0=OP.max
                        )

            def stage_B(b, i):
                """mm2 straight from h1T chunk-pairs, LN2 var, relu -> h2.

                W2 is column-centered on the host so mean_h(z2) == 0 exactly
                (up to matmul rounding): no mean subtraction, only the
                variance is needed for the deferred rstd2.
                """
                st = bag_state[b]
                ps2 = psmm.tile([P, H], f32, tag="mm", name="ps2")
                for w_, last in ((w2r, False), (w2s, True)):
                    for k in range(HK // 2):
                        nc.tensor.matmul(
                            ps2[:],
                            st["h1T"][:, 2 * k : 2 * k + 2, i * P : (i + 1) * P],
                            w_[:, 2 * k : 2 * k + 2, :],
                            start=(w_ is w2r and k == 0),
                            stop=(last and k == HK // 2 - 1),
                            perf_mode=DRm,
                        )
                bn2 = stats.tile([P, 6], f32, tag="bn", name="bn2")
                nc.vector.bn_stats(bn2[:], ps2[:])
                mv2 = stats.tile([P, 2], f32, tag="mv", name="mv2")
                nc.vector.bn_aggr(mv2[:], bn2[:])
                nc.gpsimd.tensor_copy(st["V"][:, i : i + 1], mv2[:, 1:2])
                # h2 holds the UNSCALED relu(z2); rstd2 is applied later via
                # the tanh scale and the attention weights.
                h2t = h2p.tile([P, H], bf16, tag="h2res", name="h2res")
                nc.scalar.activation(h2t[:], ps2[:], AF.Relu)
                st["h2l"][i] = h2t

            def stage_rstd(b):
                """batched rstd2 = 1/sqrt(var+eps): one ACT table swap/bag
                (no in-set table holds Sqrt together with Tanh/Exp, so the
                Sqrt must stay batched at bag boundaries)."""
                st = bag_state[b]
                sd2 = smallp.tile([P, NT], f32, tag="sd2", name="sd2")
                nc.scalar.activation(
                    sd2[:], st["V"][:], AF.Sqrt, bias=eps256_t[:], scale=1.0 / 256
                )
                R_sc = smallp.tile([P, NT], f32, tag="R", name="R_sc")
                nc.vector.reciprocal(R_sc[:], sd2[:])
                st["R"] = R_sc

            def init_small2(b):
                """softmax/pool state on 2-deep (or 1-deep) rings: allocate
                only after the PREVIOUS bag's pooling is fully emitted, or
                the ring reuse retires tiles that pooling still reads."""
                st = bag_state[b]
                st["p"] = smallp.tile([P, NT], f32, tag="p", name="p_t")
                st["attn"] = smallp.tile([P, NT], bf16, tag="attn", name="attn_t")
                st["pps"] = pspool.tile(
                    [P, HK], f32, tag="pool", name="pool_ps", bufs=1
                )

            def stage_C(b, i):
                """transpose h2 tile."""
                st = bag_state[b]
                trp2 = pstr.tile([P, H], bf16, tag="tr", name="trp2")
                h2t = st["h2l"][i]
                for c in range(HK):
                    nc.tensor.transpose(
                        trp2[:, c * P : (c + 1) * P],
                        h2t[:, c * P : (c + 1) * P],
                        ident_h[:],
                    )
                st[("trp2", i)] = trp2

            def stage_D(b, i):
                """mma, tanh (deferred rstd2 as scale), score dot."""
                st = bag_state[b]
                trp2 = st.pop(("trp2", i))
                h2T = htp.tile([P, HK, P], f8, tag="h2T", name="h2T")
                # h2 carries the 256x (W1c*16 . W2c*16) scale; shed it here so
                # fp8 stays in range (and the tanh scale R carries rstd2/16).
                if i % 2 == 0:
                    nc.scalar.activation(h2T[:], trp2[:], AF.Copy, scale=1.0 / 256)
                else:
                    nc.vector.tensor_scalar_mul(h2T[:], trp2[:], 1.0 / 256)
                # no residual pass for Wa1: its quantization error only
                # jitters per-token scores (averaged out by the softmax
                # pooling), unlike W1/W2 whose error shifts h2 systematically.
                psa = psmm.tile([P, H], f32, tag="mm", name="psa")
                for k in range(HK // 2):
                    nc.tensor.matmul(
                        psa[:],
                        h2T[:, 2 * k : 2 * k + 2, :],
                        war[:, 2 * k : 2 * k + 2, :],
                        start=(k == 0), stop=(k == HK // 2 - 1),
                        perf_mode=DRm,
                    )
                a_t = ap_.tile([P, H], bf16, tag="a", name="a_t")
                nc.scalar.activation(
                    a_t[:], psa[:], AF.Tanh, scale=st["R"][:, i : i + 1]
                )
                # wa2 multiply alternates gpsimd/DVE: gpsimd alone (1123ns)
                # paces the score phase, DVE alone overloads it — split by
                # parity. The free-axis row-sum must stay on DVE (gpsimd
                # reduces only across partitions).
                if i % 2 == 0:
                    nc.gpsimd.tensor_mul(a_t[:], a_t[:], wa2_rep_h[:])
                else:
                    nc.vector.tensor_mul(a_t[:], a_t[:], wa2_rep_h[:])
                nc.vector.tensor_reduce(
                    st["s"][:, i : i + 1], a_t[:], axis=AX.X, op=OP.add
                )

            def pool_all(b):
                """attn-weighted pooling with attn as the 1-column MOVING
                operand: ~1 cycle per matmul (vs 512 with h2 moving), and the
                output lands as [H-on-partitions, 1] chunks — already in the
                layout the classifier needs, killing the pool transpose.
                NOTE: the per-chunk accumulation groups share one PSUM bank,
                and start=True re-marks the whole bank's zero region — groups
                must run SEQUENTIALLY (all j for chunk c, then chunk c+1),
                never interleaved, or earlier chunks lose their j=0 term."""
                st = bag_state[b]
                for c in range(HK):
                    for j in range(NT):
                        nc.tensor.matmul(
                            st["pps"][:, c : c + 1],
                            st["h2l"][j][:, c * P : (c + 1) * P],
                            st["attn"][:, j : j + 1],
                            start=(j == 0), stop=(j == NT - 1),
                        )

            def stage_softmax(b):
                """batched: one exp over [P,NT] with fused row-sum, then the
                cross-partition sum, 1/Z, and the attn weights in one DVE op
                (replaces 32 tiny exps + 32 tiny muls on the ACT/DVE path)."""
                st = bag_state[b]
                zrow = stats.tile([P, 1], f32, tag="sum1", name="zrow")
                nc.scalar.activation(
                    st["p"][:], st["s"][:], AF.Exp, accum_out=zrow[:]
                )
                zsum = stats.tile([P, 1], f32, tag="nm1", name="zsum")
                nc.gpsimd.partition_all_reduce(
                    zsum[:], zrow[:], channels=P, reduce_op=bass_isa.ReduceOp.add
                )
                rz = stats.tile([P, 1], f32, tag="nm2", name="rz")
                nc.vector.reciprocal(rz[:], zsum[:])
                st["rz"] = rz
                nc.vector.tensor_mul(st["attn"][:], st["p"][:], st["R"][:])

            def stage_pool(b):
                """pooled PSUM is already [H-part, HK]: just scale by the
                global 1/Z (same value on every partition) into poolT_sb."""
                st = bag_state[b]
                # rz normalizes the softmax; the extra 1/16 sheds the last
                # power-of-2 from attn = p * (rstd2/16) against h2l's 256x.
                nc.vector.tensor_scalar(
                    poolT_sb[:, :, b], st["pps"][:], st["rz"][:], 1.0 / 16,
                    op0=OP.mult, op1=OP.mult,
                )

            cls_state = {}

            def cls_mm1(b):
                """per-bag half of the classifier's first GEMM (N=1)."""
                if "rc" not in cls_state:
                    cls_state["rc"] = psmm.tile(
                        [P, HK, NB], f32, tag="mm", name="rc_ps"
                    )
                rc = cls_state["rc"]
                for m in range(HK):
                    for k in range(HK):
                        nc.tensor.matmul(
                            rc[:, m, b : b + 1], wc1r[:, k, m, :],
                            poolT_sb[:, k, b : b + 1],
                            start=(k == 0), stop=(k == HK - 1),
                        )

            def cls_rest():
                rc = cls_state["rc"]
                rc_sb = smallp.tile([P, HK, NB], f32, tag="rc", name="rc_sb")
                for m in range(HK):
                    nc.scalar.activation(
                        rc_sb[:, m, :], rc[:, m, :], AF.Relu,
                        bias=bc1t[:, m : m + 1], scale=1.0,
                    )
                lg_ps = psmm.tile([C, NB], f32, tag="mm", name="lg_ps")
                for k in range(HK):
                    nc.tensor.matmul(
                        lg_ps[:], wc2r[:, k, :], rc_sb[:, k, :],
                        start=(k == 0), stop=(k == HK - 1),
                    )
                lg_sb = smallp.tile([C, NB], f32, tag="lg", name="lg_sb")
                nc.scalar.activation(
                    lg_sb[:], lg_ps[:], AF.Identity, bias=bc2t[:], scale=1.0
                )
                with nc.allow_non_contiguous_dma(reason="4-element logits store"):
                    nc.sync.dma_start(y.rearrange("b c -> c b"), lg_sb[:])

            def _scoped(fn, tag):
                def g(*a):
                    nm = tag + (str(a[1]) if len(a) > 1 else "")
                    with nc.named_scope(nm):
                        return fn(*a)
                return g

            stage_A = _scoped(stage_A, "A")
            stage_B = _scoped(stage_B, "B")
            stage_C = _scoped(stage_C, "C")
            stage_D = _scoped(stage_D, "D")
            stage_rstd = _scoped(stage_rstd, "rstd")
            stage_softmax = _scoped(stage_softmax, "sm")
            stage_pool = _scoped(stage_pool, "pool")
            prefetch_super = _scoped(prefetch_super, "pf")

            def init_bag(b):
                st = bag_state[b]
                st["h2l"] = {}
                st["s"] = smallp.tile([P, NT], f32, tag="s", name="s_sc")
                st["V"] = smallp.tile([P, NT], f32, tag="V", name="V_sc")
                st["h1T"] = h1tp.tile([P, HK, N], f8, tag="h1T", name="h1T8")

            if fast:
                # Cross-bag phase overlap: bag b's CD phase (ACT/DVE/Pool
                # heavy, PE light) runs interleaved with bag b+1's AB phase
                # (PE heavy) in one "brace" loop. Both phases use only
                # exp_and_others ACT functions (Tanh/Exp/Relu/Copy), so the
                # merged stream needs NO activation-table swaps; the Sqrt
                # (its own table set) stays batched at bag boundaries.
                # Within an iteration, ready work (CD of bag b) is emitted
                # before fresh-dependency work (AB of b+1) — the engines'
                # in-order queues head-of-line-block otherwise.
                # ---- bag 0 AB phase, solo ----
                init_bag(0)
                prefetch_super(0, 0)
                emit_w1(0, DK)
                emit_w1s()
                prefetch_super(0, 1)
                for i in range(NT):
                    if i % XS == 0:
                        s = i // XS
                        if s + 2 < SUP:
                            prefetch_super(0, s + 2)
                        stage_A(0, s)
                    if i == 1:
                        emit_w2()
                    if i == 2:
                        emit_wa1()
                    if i == 8:
                        emit_wcls()
                    if i >= 2:
                        stage_B(0, i - 2)
                stage_B(0, NT - 2)
                stage_B(0, NT - 1)
                stage_rstd(0)
                init_small2(0)
                # ---- braces: CD(b) interleaved with AB(b+1) ----
                for b in range(NB):
                    nb_ = b + 1
                    if nb_ < NB:
                        init_bag(nb_)
                        prefetch_super(nb_, 0)
                        prefetch_super(nb_, 1)
                    for i in range(NT):
                        if i >= 2:
                            stage_D(b, i - 2)
                        stage_C(b, i)
                        if nb_ < NB:
                            if i % XS == 0:
                                s = i // XS
                                if s + 2 < SUP:
                                    prefetch_super(nb_, s + 2)
                                stage_A(nb_, s)
                            if i >= 2:
                                stage_B(nb_, i - 2)
                    stage_D(b, NT - 2)
                    stage_D(b, NT - 1)
                    if nb_ < NB:
                        stage_B(nb_, NT - 2)
                        stage_B(nb_, NT - 1)
                        stage_rstd(nb_)
                    else:
                        cls_mm1(0)
                    stage_softmax(b)
                    pool_all(b)
                    stage_pool(b)
                    if nb_ < NB:
                        init_small2(nb_)
            else:
                for b in range(NB):
                    h2_res = h2p.tile(
                        [P, NT, H], f32r, tag="h2big", name="h2res", bufs=1
                    )
                    s_sc = smallp.tile([P, NT], f32, tag="s", name="s_sc")

                    for i in range(NT):
                        xt_r = xtr.tile([P, DK, P], f32r, tag="xtr", name="xt_r")
                        nc.sync.dma_start(
                            xt_r[:], xt_part[b, :, :, i * P : (i + 1) * P]
                        )
                        ps1 = psmm.tile([P, H], f32, tag="mm", name="ps1")
                        for k in range(DK):
                            nc.tensor.matmul(
                                ps1[:], xt_r[:, k, :], w1r[:, k, :],
                                start=(k == 0), stop=(k == DK - 1),
                            )
                        h1 = h1p.tile([P, H], f32r, tag="h1", name="h1")
                        ln_relu(ps1, h1[:], b1_rep, g1_rep, be1_rep, "1")

                        trp1 = pstr.tile([P, H], f32r, tag="tr", name="trp1")
                        for c in range(HK):
                            nc.tensor.transpose(
                                trp1[:, c * P : (c + 1) * P],
                                h1[:, c * P : (c + 1) * P],
                                ident_r[:],
                            )
                        h1T = htp.tile([P, HK, P], f32r, tag="h1T", name="h1T")
                        nc.scalar.copy(h1T[:], trp1[:])

                        ps2 = psmm.tile([P, H], f32, tag="mm", name="ps2")
                        for k in range(HK):
                            nc.tensor.matmul(
                                ps2[:], h1T[:, k, :], w2r[:, k, :],
                                start=(k == 0), stop=(k == HK - 1),
                            )
                        ln_relu(ps2, h2_res[:, i, :], b2_rep, g2_rep, be2_rep, "2")

                        trp2 = pstr.tile([P, H], f32r, tag="tr", name="trp2")
                        for c in range(HK):
                            nc.tensor.transpose(
                                trp2[:, c * P : (c + 1) * P],
                                h2_res[:, i, c * P : (c + 1) * P],
                                ident_r[:],
                            )
                        h2T = htp.tile([P, HK, P], f32r, tag="h2T", name="h2T")
                        nc.vector.tensor_copy(h2T[:], trp2[:])

                        psa = psmm.tile([P, H], f32, tag="mm", name="psa")
                        for k in range(HK):
                            nc.tensor.matmul(
                                psa[:], h2T[:, k, :], war[:, k, :],
                                start=(k == 0), stop=(k == HK - 1),
                            )
                        a_t = ap_.tile([P, H], f32, tag="a", name="a_t")
                        if ba1_rep is not None:
                            nc.vector.tensor_add(a_t[:], psa[:], ba1_rep[:])
                            nc.scalar.activation(a_t[:], a_t[:], AF.Tanh)
                        else:
                            nc.scalar.activation(a_t[:], psa[:], AF.Tanh)
                        nc.gpsimd.tensor_mul(a_t[:], a_t[:], wa2_rep[:])
                        nc.vector.tensor_reduce(
                            s_sc[:, i : i + 1], a_t[:], axis=AX.X, op=OP.add
                        )

                    rmax = stats.tile([P, 1], f32, tag="sum1", name="rmax")
                    nc.vector.tensor_reduce(rmax[:], s_sc[:], axis=AX.X, op=OP.max)
                    gmax = stats.tile([P, 1], f32, tag="nm1", name="gmax")
                    nc.gpsimd.partition_all_reduce(
                        gmax[:], rmax[:], channels=P, reduce_op=bass_isa.ReduceOp.max
                    )
                    ngmax = stats.tile([P, 1], f32, tag="nm2", name="ngmax")
                    nc.vector.tensor_scalar_mul(ngmax[:], gmax[:], -1.0)
                    p_t = smallp.tile([P, NT], f32, tag="p", name="p_t")
                    zrow = stats.tile([P, 1], f32, tag="sum1", name="zrow")
                    nc.scalar.activation(
                        p_t[:], s_sc[:], AF.Exp, bias=ngmax[:], scale=1.0,
                        accum_out=zrow[:],
                    )
                    zsum = stats.tile([P, 1], f32, tag="nm1", name="zsum")
                    nc.gpsimd.partition_all_reduce(
                        zsum[:], zrow[:], channels=P, reduce_op=bass_isa.ReduceOp.add
                    )
                    rz = stats.tile([P, 1], f32, tag="nm2", name="rz")
                    nc.vector.reciprocal(rz[:], zsum[:])
                    attn_t = smallp.tile([P, NT], f32r, tag="attn", name="attn_t")
                    nc.vector.tensor_scalar_mul(attn_t[:], p_t[:], rz[:])

                    pool_ps = psmm.tile([1, H], f32, tag="mm", name="pool_ps")
                    for i in range(NT):
                        nc.tensor.matmul(
                            pool_ps[:], attn_t[:, i : i + 1], h2_res[:, i, :],
                            start=(i == 0), stop=(i == NT - 1),
                        )
                    pooled_sb = smallp.tile([P, H], f32, tag="pooled", name="pooled_sb")
                    nc.vector.memset(pooled_sb[:], 0.0)
                    nc.vector.tensor_copy(pooled_sb[0:1, :], pool_ps[:])
                    poolT_ps = pstr.tile([P, H], f32, tag="tr", name="poolT_ps")
                    for c in range(HK):
                        nc.tensor.transpose(
                            poolT_ps[:, c * P : (c + 1) * P],
                            pooled_sb[:, c * P : (c + 1) * P],
                            ident_f[:],
                        )
                    nc.vector.tensor_copy(
                        poolT_sb[:, :, b],
                        poolT_ps.rearrange("p (c j) -> p c j", j=P)[:, :, 0],
                    )

            # ---- classifier tail ----
            if fast:
                cls_mm1(NB - 1)
                cls_rest()
            else:
                cls_mm1(0)
                cls_mm1(NB - 1)
                cls_rest()

    nc.compile()
    return nc


def _get_program(flags):
    if flags not in _BUILD_CACHE:
        _BUILD_CACHE[flags] = _build(flags)
    return _BUILD_CACHE[flags]


def kernel(**inputs):
    import sys
    for pth in ("/opt/trn_rl_repo",):
        if pth not in sys.path:
            sys.path.append(pth)
    from concourse.bass_utils import run_bass_kernel_spmd

    x = np.asarray(inputs["x"], dtype=np.float32)
    names = ["W1", "b1", "g1", "beta1", "W2", "b2", "g2", "beta2",
             "Wa1", "ba1", "wa2", "ba2", "Wc1", "bc1", "Wc2", "bc2"]
    w = {k: np.asarray(inputs[k], dtype=np.float32) for k in names}

    z_b1 = bool((w["b1"] == 0).all())
    aff1 = not bool((w["g1"] == 1).all() and (w["beta1"] == 0).all())
    z_b2 = bool((w["b2"] == 0).all())
    aff2 = not bool((w["g2"] == 1).all() and (w["beta2"] == 0).all())
    z_ba1 = bool((w["ba1"] == 0).all())
    # no-max-shift softmax is safe iff scores can't overflow exp in fp32
    safe_exp = bool(np.abs(w["wa2"]).sum() < 60.0)
    flags = (z_b1, aff1, z_b2, aff2, z_ba1, safe_exp)
    fast = z_b1 and z_b2 and z_ba1 and not aff1 and not aff2 and safe_exp
    if fast:
        # Column-center W1/W2: (x @ W1c) == x@W1 - mean_h(x@W1), making the
        # LayerNorm mean subtractions free on-device. Weights ship as
        # fp8e4m3 scaled by 16 (x ~N(0,1) needs no scale); the power-of-2
        # scales cancel exactly through the scale-invariant LN chain, the
        # deferred rstd2, and the final pooled normalization.
        import ml_dtypes
        f8 = ml_dtypes.float8_e4m3
        w = dict(w)

        def split_f8(m):
            hi = m.astype(f8)
            res = (m - hi.astype(np.float32)).astype(f8)
            return hi, res

        w["W1"], w["W1s"] = split_f8(
            (w["W1"] - w["W1"].mean(axis=1, keepdims=True)) * 16
        )
        w["W2"], w["W2s"] = split_f8(
            (w["W2"] - w["W2"].mean(axis=1, keepdims=True)) * 16
        )
        w["Wa1"], w["Wa1s"] = split_f8(w["Wa1"] * 16)

    nc = _get_program(flags)

    in_maps = []
    for core in range(NCORES):
        shard = x[core * NB : (core + 1) * NB]          # [NB, N, D]
        xtr = np.ascontiguousarray(shard.transpose(0, 2, 1))  # [NB, D, N]
        if fast:
            import ml_dtypes
            xtr = xtr.astype(ml_dtypes.float8_e4m3)
        m = {
            "xt": xtr,
            "W1": w["W1"], "W2": w["W2"], "Wa1": w["Wa1"],
            "wa2": w["wa2"].reshape(H, 1),
            "Wc1": w["Wc1"], "Wc2": w["Wc2"],
            "bc1": w["bc1"], "bc2": w["bc2"],
        }
        if fast:
            m["W1s"] = w["W1s"]
            m["W2s"] = w["W2s"]
            m["Wa1s"] = w["Wa1s"]
        if not z_b1:
            m["b1"] = w["b1"]
        if aff1:
            m["g1"] = w["g1"]
            m["beta1"] = w["beta1"]
        if not z_b2:
            m["b2"] = w["b2"]
        if aff2:
            m["g2"] = w["g2"]
            m["beta2"] = w["beta2"]
        if not z_ba1:
            m["ba1"] = w["ba1"]
        in_maps.append(m)

    res = run_bass_kernel_spmd(nc, in_maps, core_ids=list(range(NCORES)))
    out = np.concatenate([res.results[i]["y"] for i in range(NCORES)], axis=0)
    return out.astype(np.float32)

